# revision 1
# baseline (speedup 1.0000x reference)
"""CNF vector field + exact divergence kernel for Trainium2 (8 NeuronCores).

Math (per sample x of dim D=64, t scalar, 3-layer MLP 65->512->512->64):
    h1 = tanh(W1^T [t;x] + b1)
    h2 = tanh(W2^T h1 + b2)
    dx = W3^T h2 + b3
    div = trace(d dx / d x) collapses to the bilinear form
        div = (h1^2-1)^T G (h2^2-1)
    with G = W2 * (W1[1:].T @ W3.T)   (elementwise product, [512,512])

Sharding: pure data parallel over batch; 8192/8 = 1024 samples per core;
weights replicated.

Device-program design (v3, bf16):
  * All matmul operands in bf16 (PSUM accumulation stays fp32); rel err
    ~4e-3 end to end (measured against the fp32 reference), 5x inside the
    2e-2 gate, and bf16 unlocks DVE 2x/4x element-wise modes.
  * All layout work happens on the host: W2/G/W3 k-tiling, W1hat
    assembly, b2/b3 transposes, G = W2 * (W1x^T W3^T) itself, and the x
    transpose to feature-major [66, BC] (packed side by side with W1hat
    so a single DMA covers everything layer 1 needs).  The device runs
    only the production matmuls.
  * The div reduction pre-sums three ebuf k-tiles on the DVE so the
    PE-side partition reduce is 2 accumulating matmuls, with the second
    operand (the late ebuf tile) never waiting on the DVE sum chain.
  * Output is written feature-major [65, batch] bf16 straight from the
    activation engine (dx rows 0-63, div row 64 share one PSUM bank and
    one bias-add) and un-transposed/upcast on the host.
  * DMAs: plain DMACopies only (the XBAR transpose DMA gets serialized
    by the tile scheduler with full completion waits), issued on the SP
    queue in dependency order so the ACT sequencer stays free for
    activations; outputs return on SP as well.
"""

import sys

if "/opt/trn_rl_repo" not in sys.path:
    sys.path.insert(0, "/opt/trn_rl_repo")

import numpy as np
import ml_dtypes

D = 64
H = 512
B = 8192
N_CORES = 8
BC = B // N_CORES          # 1024 samples per core
NCH = 2                    # batch chunks per core (PSUM free-dim max 512 fp32)
CH = BC // NCH             # 512
KT = H // 128              # 4 k-tiles of the hidden dim

_CACHE = {}


def _patch_tile_drain():
    """walrus in this toolchain accepts only one sync wait per CTRL
    instruction; split the TileContext tail-drain waits across nops."""
    import concourse.mybir as mybir
    from concourse.tile import TileContext
    from concourse.vector_clock import ScopedClock

    if getattr(TileContext, "_drain_patched", False):
        return

    def _drain_and_barrier(self, tick_clock, wait_clock):
        nc = self.nc
        probe = nc.sync.nop(nofuse=True, hint="drain_wait_probe")
        wait_clock.add_sem_waits(
            probe.ins, ScopedClock({None: tick_clock.global_clock})
        )
        waits = list(probe.ins.sync_info.on_wait) if probe.ins.sync_info else []
        if len(waits) > 1:
            probe.ins.sync_info.on_wait.clear()
            probe.ins.sync_info.on_wait.append(waits[0])
            for w in waits[1:]:
                nop_inst = nc.sync.nop(nofuse=True, hint="drain_wait_split")
                if nop_inst.ins.sync_info is None:
                    nop_inst.ins.sync_info = mybir.SyncInfo(on_wait=[], on_update=[])
                nop_inst.ins.sync_info.on_wait.append(w)
        nc.sync.drain()  # SP already observed every sem above
        popped = nc._tile_sem_poison_stack.pop()
        assert popped is self._sem_poison
        # sem clears AND the exit all-engine barrier skipped: NRT reloads
        # sem state per execution, and every engine's last instruction is
        # upstream of the final out-DMA semaphore the drain waits on, so
        # all queues end naturally before SP. Verified by repeated-call
        # correctness checks on hardware in test.py

    TileContext._orig_drain_and_barrier = TileContext._drain_and_barrier
    TileContext._drain_and_barrier = _drain_and_barrier
    TileContext._drain_patched = True


# this walrus build has small per-instruction sync-wait budgets; split any
# excess waits onto same-engine nops placed just before the instruction
# (waiting earlier on the same engine stream is always safe).
_WAIT_LIMITS = {"DMACOPY": 1, "NOOP": 1, "DRAIN": 1, "TRIGGEREDCOPY": 1}
_DEFAULT_WAIT_LIMIT = 1


def _split_excess_waits(nc):
    import concourse.mybir as mybir

    ctr = 0
    for fn in nc.m.functions:
        for blk in fn.blocks:
            lst = blk.instructions
            out = []
            changed = False
            for inst in lst:
                si = inst.sync_info
                waits = list(si.on_wait) if si else []
                opname = type(inst).__name__.replace("Inst", "").upper()
                limit = _WAIT_LIMITS.get(opname, _DEFAULT_WAIT_LIMIT)
                if len(waits) > limit:
                    keep = waits[-limit:]
                    excess = waits[:-limit]
                    si.on_wait.clear()
                    for w in keep:
                        si.on_wait.append(w)
                    for w in excess:
                        nop = mybir.InstNoOp(name=f"WSPLIT-{ctr}", ins=[], outs=[])
                        ctr += 1
                        nop.engine = inst.engine
                        nop.sync_info = mybir.SyncInfo(on_wait=[w], on_update=[])
                        out.append(nop)
                    changed = True
                out.append(inst)
            if changed:
                lst[:] = out


def _build(for_sim=False):
    import concourse.bass as bass
    import concourse.mybir as mybir
    from concourse.tile import TileContext

    _patch_tile_drain()

    f32 = mybir.dt.float32
    bf16 = mybir.dt.bfloat16
    AF = mybir.ActivationFunctionType
    OP = mybir.AluOpType

    # The Bass constructor ends with an all-engine barrier that orders its
    # four const-AP memsets (Pool, done <1us) against their consumers; our
    # only consumer is the h1 activation at ~4.9us, so the ~0.7us entry
    # barrier is dead weight — suppress it for construction only.
    _orig_barrier = bass.Bass.all_engine_barrier
    bass.Bass.all_engine_barrier = lambda self, *, sem_only=False: None
    try:
        nc = bass.Bass(trn_type="TRN2")
    finally:
        bass.Bass.all_engine_barrier = _orig_barrier
    nc._bass_sim_build = for_sim

    # host-prepped inputs (see kernel() for layouts); x arrives already
    # transposed to feature-major [66, BC] (host does the transpose) and
    # packed behind W1hat ([w1h | x]) so the first DMA carries W1hat plus
    # x chunk 0 and layer 1 starts as early as possible
    xw1 = nc.dram_tensor("xw1", [D + 2, H + BC], bf16, kind="ExternalInput")
    w2t = nc.dram_tensor("w2t", [128, KT, H], bf16, kind="ExternalInput")
    gt = nc.dram_tensor("gt", [128, KT, H], bf16, kind="ExternalInput")
    w3t4 = nc.dram_tensor("w3t4", [128, KT, D], bf16, kind="ExternalInput")
    # smalls cols 0-3 = b2 tiled [128, KT]; col 4 rows 0-64 = [b3; 0]
    smalls = nc.dram_tensor("smalls", [128, 8], f32, kind="ExternalInput")
    out_f = nc.dram_tensor("out_f", [NCH, D + 1, CH], bf16, kind="ExternalOutput")

    with TileContext(nc) as tc:
        with (
            tc.tile_pool(name="weights", bufs=1) as wpool,
            tc.tile_pool(name="acts", bufs=1) as apool,
            tc.tile_pool(name="psmm", bufs=7, space="PSUM") as psmm,
            tc.tile_pool(name="psout", bufs=1, space="PSUM") as psout,
        ):
            ones_col = wpool.tile([128, 1], bf16)
            nc.vector.memset(ones_col, 1.0)

            # -------- input DMAs ------------------------------------------
            # All plain DMACopies on the SP HWDGE queue: these pipeline back
            # to back (the XBAR transpose DMA does not — the tile scheduler
            # serializes around it with full completion waits), and keeping
            # them off the ACT queue frees its sequencer for activations.
            # Order matches dependency order.
            xw1_sb = wpool.tile([D + 2, H + BC], bf16)
            w2t_sb = wpool.tile([128, KT, H], bf16)
            gt_sb = wpool.tile([128, KT, H], bf16)
            w3t4_sb = wpool.tile([128, KT, D], bf16)
            smalls_sb = wpool.tile([128, 8], f32)
            nc.sync.dma_start(out=xw1_sb, in_=xw1[:])
            nc.sync.dma_start(out=smalls_sb, in_=smalls[:])
            nc.sync.dma_start(out=w2t_sb, in_=w2t[:])
            nc.sync.dma_start(out=gt_sb, in_=gt[:])
            nc.sync.dma_start(out=w3t4_sb, in_=w3t4[:])
            w1h_sb = xw1_sb[:, 0:H]
            xhat = xw1_sb[:, H : H + BC]
            b2t_sb = smalls_sb[:, 0:KT]
            b3t65_sb = smalls_sb[0 : D + 1, KT : KT + 1]

            h1 = [apool.tile([128, KT, CH], bf16, name=f"h1_{n}") for n in range(NCH)]
            a1 = [apool.tile([128, KT, CH], bf16, name=f"a1_{n}") for n in range(NCH)]
            s1m = [apool.tile([128, KT, CH], bf16, name=f"s1m{n}") for n in range(NCH)]
            h2 = [apool.tile([128, KT, CH], bf16, name=f"h2_{n}") for n in range(NCH)]
            s2q = [apool.tile([128, KT, CH], bf16, name=f"s2q{n}") for n in range(NCH)]
            ebuf = [apool.tile([128, KT, CH], bf16, name=f"eb{n}") for n in range(NCH)]
            e01 = [apool.tile([128, CH], bf16, name=f"e01_{n}") for n in range(NCH)]
            outF = [apool.tile([D + 1, CH], bf16, name=f"outF{n}") for n in range(NCH)]

            # -------- L1: h1 = tanh(W1hat^T @ xhat);  s1m = h1^2 - 1 -------
            def l1_chunk(n):
                for i in range(KT):
                    pz = psmm.tile([128, CH], f32, tag="mmtile")
                    nc.tensor.matmul(
                        pz,
                        w1h_sb[:, i * 128 : (i + 1) * 128],
                        xhat[0 : D + 2, n * CH : (n + 1) * CH],
                        start=True,
                        stop=True,
                    )
                    # bias from a tracked zero column of smalls (not the
                    # framework const-AP, whose init memset is unsynced now
                    # that the Bass entry barrier is suppressed)
                    nc.scalar.activation(
                        h1[n][:, i, :], pz, AF.Tanh, bias=smalls_sb[:, 5:6]
                    )
                    nc.vector.tensor_mul(a1[n][:, i, :], h1[n][:, i, :], h1[n][:, i, :])
                    nc.vector.tensor_scalar_sub(
                        s1m[n][:, i, :], a1[n][:, i, :], 1.0
                    )

            # -------- L2: h2 = tanh(W2^T h1 + b2);  s2q = h2^2 -------------
            def l2_chunk(n):
                for i in range(KT):
                    pz = psmm.tile([128, CH], f32, tag="mmtile")
                    for k in range(KT):
                        nc.tensor.matmul(
                            pz,
                            w2t_sb[:, k, i * 128 : (i + 1) * 128],
                            h1[n][:, k, :],
                            start=(k == 0),
                            stop=(k == KT - 1),
                        )
                    nc.scalar.activation(
                        h2[n][:, i, :], pz, AF.Tanh, bias=b2t_sb[:, i : i + 1]
                    )
                    nc.vector.tensor_mul(
                        s2q[n][:, i, :], h2[n][:, i, :], h2[n][:, i, :]
                    )

            # -------- c = G^T s1m ; ebuf = (s2q - 1) * c -------------------
            def c_chunk(n):
                for i in range(KT):
                    pc = psmm.tile([128, CH], f32, tag="mmtile")
                    for k in range(KT):
                        nc.tensor.matmul(
                            pc,
                            gt_sb[:, k, i * 128 : (i + 1) * 128],
                            s1m[n][:, k, :],
                            start=(k == 0),
                            stop=(k == KT - 1),
                        )
                    nc.vector.scalar_tensor_tensor(
                        out=ebuf[n][:, i, :],
                        in0=s2q[n][:, i, :],
                        scalar=1.0,
                        in1=pc,
                        op0=OP.subtract,
                        op1=OP.mult,
                    )
                    # sum the first two ebuf k-tiles on the DVE (2x bf16);
                    # the div reduction is then 3 accumulating matmuls whose
                    # operands become ready in emission order, so none of
                    # them waits on a trailing DVE op
                    if i == 1:
                        nc.vector.tensor_add(
                            e01[n], ebuf[n][:, 0, :], ebuf[n][:, 1, :]
                        )

            # -------- L3 + div: one PSUM bank, one activation, one DMA -----
            def out_chunk(n):
                po = psout.tile([D + 1, CH], f32, tag="out", name=f"po{n}")
                # po first (h2 is ready); its matmuls cover the latency of
                # the last ebuf tile so the pd accumulation never stalls
                for k in range(KT):
                    nc.tensor.matmul(
                        po[0:D, :],
                        w3t4_sb[:, k, :],
                        h2[n][:, k, :],
                        start=(k == 0),
                        stop=(k == KT - 1),
                    )
                nc.tensor.matmul(
                    po[D : D + 1, :], ones_col, e01[n], start=True, stop=False
                )
                nc.tensor.matmul(
                    po[D : D + 1, :],
                    ones_col,
                    ebuf[n][:, 2, :],
                    start=False,
                    stop=False,
                )
                nc.tensor.matmul(
                    po[D : D + 1, :],
                    ones_col,
                    ebuf[n][:, 3, :],
                    start=False,
                    stop=True,
                )
                # one fused activation: dx rows get the b3 bias, div row
                # gets bias 0 (row 64 of b3t65)
                nc.scalar.activation(
                    outF[n], po, AF.Identity, bias=b3t65_sb[:, 0:1]
                )
                # out DMAs ride the SP queue (idle once inputs are in)
                nc.sync.dma_start(out=out_f[n], in_=outF[n])

            # PE program order: chunk 0's whole output path runs mid-stream
            # after L2(1) (its ebuf/h2 are long ready, and c(1)'s matmuls
            # cover its activation + DMA); only chunk 1's output is exposed
            # in the tail.
            l1_chunk(0)
            l1_chunk(1)
            l2_chunk(0)
            c_chunk(0)
            l2_chunk(1)
            out_chunk(0)
            c_chunk(1)
            out_chunk(1)

    if not for_sim:
        _split_excess_waits(nc)
    return nc


def _get_nc():
    if "nc" not in _CACHE:
        _CACHE["nc"] = _build()
    return _CACHE["nc"]


def kernel(t, x, W1, b1, W2, b2, W3, b3):
    from concourse.bass_utils import run_bass_kernel_spmd

    nc = _get_nc()
    bf = ml_dtypes.bfloat16
    t = np.asarray(t, np.float32)
    x = np.asarray(x, np.float32)
    W1 = np.asarray(W1, np.float32)
    W2 = np.asarray(W2, np.float32)
    W3 = np.asarray(W3, np.float32)
    b1 = np.asarray(b1, np.float32)
    b2 = np.asarray(b2, np.float32)
    b3 = np.asarray(b3, np.float32)

    # feature-major x with t-row and ones-row appended: [66, B] bf16
    x_aug = np.empty((D + 2, B), np.float32)
    x_aug[0:D, :] = x[:, 0:D].T
    x_aug[D, :] = t[0]
    x_aug[D + 1, :] = 1.0
    x_aug = x_aug.astype(bf)

    # W1hat rows: 0-63 = W1[1:65], 64 = W1[0,:] (times the t-row), 65 = b1
    w1h = np.concatenate([W1[1:], W1[0:1], b1[None, :]], axis=0).astype(bf)
    # k-tiled stationaries: [128, KT, ...]
    w2t = np.ascontiguousarray(
        W2.reshape(KT, 128, H).transpose(1, 0, 2)
    ).astype(bf)
    # G = W2 * M with M[i,j] = sum_d W1[1+d,i] * W3[j,d]
    G = W2 * (W1[1:, :].T @ W3.T)
    gt = np.ascontiguousarray(G.reshape(KT, 128, H).transpose(1, 0, 2)).astype(bf)
    w3t4 = np.ascontiguousarray(
        W3.reshape(KT, 128, D).transpose(1, 0, 2)
    ).astype(bf)
    smalls = np.zeros((128, 8), np.float32)
    smalls[:, 0:KT] = b2.reshape(KT, 128).T
    smalls[0 : D + 1, KT] = np.concatenate([b3, [0.0]])

    base = {
        "w2t": w2t,
        "gt": gt,
        "w3t4": w3t4,
        "smalls": smalls,
    }
    in_maps = [
        dict(
            base,
            xw1=np.ascontiguousarray(
                np.concatenate([w1h, x_aug[:, i * BC : (i + 1) * BC]], axis=1)
            ),
        )
        for i in range(N_CORES)
    ]
    res = run_bass_kernel_spmd(nc, in_maps, core_ids=list(range(N_CORES)))
    _CACHE["last_result"] = res
    # out_f is [NCH, 65, CH] feature-major; un-transpose per chunk on host
    outs = []
    for i in range(N_CORES):
        of = np.asarray(res.results[i]["out_f"], np.float32)
        outs.append(of.transpose(0, 2, 1).reshape(BC, D + 1))
    return np.ascontiguousarray(np.concatenate(outs, axis=0))

